# revision 35
# baseline (speedup 1.0000x reference)
"""MQA attention kernel for Trainium2, sharded over 8 NeuronCores.

Problem: query [1, 2048, 16, 128] f32, shared key/value [1, 2048, 128] f32,
mask [1, 16, 2048, 2048] bool (all ones -> no-op, per problem spec fill).

Sharding: tensor-parallel over heads, 2 heads per core; K/V replicated.

Per-core roofline: 65536 exp-elements/lane on ScalarE (54.6us at 1.2GHz) and
~131.6k matmul stream cycles on the PE (54.8us at 2.4GHz) -- co-critical.
The schedule keeps the ScalarE exp stream dense from ~9us to the end:

  - scores are computed as 128-col "rects" S^T[kv 128, q 128] (one fp16
    matmul each; 128-col matmuls sustain ~60ns incl hidden LDWEIGHTS).
    Rect order is chosen for DMA arrival + PV readiness:
      phase A: q[0:512] x kv tiles 0-7   (only needs the two upfront DMAs)
      phase B: q[0:512] x kv tiles 8-15  (needs kT tail, arrives ~12.4us)
      phase C: per 128-q chunk, all 16 kv tiles (chunk becomes PV-ready
               ~1.7us after its columns are exp'd -> small structural tail)
  - rects pack into [128, <=1536] PSUM tiles (3 banks, double-buffered);
    ONE ScalarE Exp per tile; ramp groups are small ([1,2,3,6] rects) so
    the exp stream starts as soon as the first 64KB DMA lands (~9us).
  - PV: out[q,0:128]+denominator in one PSUM accumulation group per 128-q
    chunk (lhsT = pT piece stationary, rhs = [V | ones] moving). PV pops
    are gated LAG=2 groups behind the exp stream (lag shrinking to 0 at
    the end): backlog drains into PE slack and overlaps the final acts.
  - input DMAs all issue upfront on the sync HWDGE ring in need-order;
    the ring starts transfers in issue order so the tiny act0 transfers
    aren't slowed by the bulk ones behind them.
  - 9 PE warmup matmuls + filler matmuls woven between the ramp groups
    keep the HAM activity window hot so the 2.4GHz clock gate opens
    ~10.5us in (it slipped to 18us without the fillers).
  - a custom-DVE exp path (EXPB cubic base + SQ4 squarings) is registered
    and was validated numerically, but left disabled (DVE_G = set()):
    the PE's scores+PV streams (~70us with per-matmul overheads) sit
    right against ScalarE's exp stream, so offloading exp just moves the
    bottleneck (measured 90us vs 81.7us ScalarE-only).

Host side: pre-transposes Q/K (free on CPU), casts to fp16, appends the
ones column to V, scatters per-core inputs, gathers per-core outputs.
"""

import numpy as np

import concourse.bass as bass
import concourse.tile as tile
from concourse import bacc, mybir
from concourse.bass_utils import run_bass_kernel_spmd

N_CORES = 8
H = 16
HPC = H // N_CORES   # heads per core
Q = 2048
KV = 2048
D = 128
P = 128
NKV = KV // P        # 16 kv tiles
VA = D + 1           # V augmented with a ones column
QTOT = HPC * Q       # q columns per core (across its heads)
NCH = QTOT // P      # 32 output q-chunks per core
SCALE = float(1.0 / np.sqrt(np.float32(D)))

ACT_FD = 1536        # one activation instruction per [128, ACT_FD] PSUM tile
RAMP_PACK = [1, 2, 3, 6]   # rects per act group during the ramp
LAG = 2              # PV pops trail the exp stream by this many groups
RAMP_MIN_G = 5       # no PV pops before this many acts are emitted

# store blocks, in 128-q chunks: the final block is a lone chunk so the
# last evac->store->receipt chain is as short as possible
STORE_BLK = [4, 4, 4, 4, 4, 4, 4, 3, 1]

F32 = mybir.dt.float32
F16 = mybir.dt.float16

# --- custom DVE exp -------------------------------------------------------
# exp(x*SCALE) ~= ((u+R*s)(u^2 + a*s*u + b*s^2))^16 with u = x*SCALE*s/16,
# s = 6^(-1/3) folding away the Taylor 1/6: the inner cubic is the factored
# degree-3 Taylor of e^u. Max rel err 1.2e-2 at |logit|=5.5, <1e-3 for
# |logit|<=3 (99.7% of N(0,1) mass); softmax-level impact ~3e-4.
# Runs as two DVE instructions: EXPB (6 ALU stages, PSUM->SBUF fp32) and
# SQ4 (4 stages of squaring, SBUF->SBUF fp16). This offloads ~20% of the
# exp stream from ScalarE (the kernel's critical path) to the idle DVE.
_DVE_S = 6.0 ** (-1.0 / 3.0)
EXPB_C0 = float(SCALE * _DVE_S / 16.0)
EXPB_C1 = 0.8783520721075015    # R*s
EXPB_C2 = 0.7726115523398118    # a*s
EXPB_C3 = 1.1384956349002726    # b*s^2

_CACHE = {}


def _register_dve_ops():
    if "dve" in _CACHE:
        return _CACHE["dve"]
    import concourse.dve_ops as dve_ops_mod
    from concourse.dve_ops import OPS, DveOp, get_dve_sub_opcode
    from concourse.dve_spec import (
        Spec, Src0, C0, C1, C2, C3, sq, lower, _spill_c3_to_src1, _has_src1,
    )
    from concourse.dve_uop import DveOpSpec

    existing = {op.name: op for op in OPS}
    if "ANT_EXPB" in existing:
        ops = (existing["ANT_EXPB"], existing["ANT_SQ4"])
    else:
        u = Src0 * C0
        body = _spill_c3_to_src1((u + C1) * (u * (u + C2) + C3))
        expb = DveOp("ANT_EXPB", Spec(body=body), subdim=False, uops_sha={})
        sq4 = DveOp("ANT_SQ4", Spec(body=sq(sq(sq(sq(Src0))))), subdim=False,
                    uops_sha={})
        for op in (expb, sq4):
            OPS.append(op)
            dve_ops_mod._SUB_OPCODE_FOR_NAME[op.name] = (
                dve_ops_mod._CUSTOM_DVE_ROW_BASE + len(OPS) - 1)
            assert dve_ops_mod._SUB_OPCODE_FOR_NAME[op.name] < 0x20
            for ver in ("v3", "v4"):
                compiled = DveOpSpec(
                    name=op.name, opcode=get_dve_sub_opcode(op.name),
                    uops=lower(op.spec, ver=ver), rd1_en=_has_src1(op.spec))
                op.uops_sha[ver] = compiled.sha(ver)
        ops = (expb, sq4)
    _CACHE["dve"] = ops
    return ops


def _plan():
    """Static schedule.

    Returns:
      groups: list of act groups; each is a list of rects (qb, i, off) with
              off the rect's column offset inside the group's PSUM tile.
      loc:    (i, chunk j) -> (g, off) location of that pT piece.
      """
    rects = []  # (qb, i, w)
    # phase A1: i 0..1 x q[0:512] (kT tiles 0-1 + q[0:512] arrive first)
    for i in range(2):
        for qb in range(0, 512, P):
            rects.append((qb, i, P))
    # phase A2: i 2..7 x q[0:512]
    for i in range(2, 8):
        for qb in range(0, 512, P):
            rects.append((qb, i, P))
    # phase B: qb-major so chunks 0-3 become PV-complete early and
    # staggered (avoids a mid-stream PV pileup); kT tail lands ~11.5us
    for qb in range(0, 512, P):
        for i in range(8, 16):
            rects.append((qb, i, P))
    # phase C1: 256-wide rects (halves the PE per-matmul overhead; each
    # rect covers 2 chunks, so PV readiness still arrives in small bursts)
    for qb in range(512, QTOT - 512, 2 * P):
        for i in range(NKV):
            rects.append((qb, i, 2 * P))
    # phase C2: last 4 chunks at 128 so the final PV/evac chain is short
    for qb in range(QTOT - 512, QTOT, P):
        for i in range(NKV):
            rects.append((qb, i, P))
    assert sum(w for (_, _, w) in rects) == QTOT * NKV

    groups = []
    loc = {}
    k = 0
    ramp = list(RAMP_PACK)
    while k < len(rects):
        grp = []
        fd = 0
        limit = ramp.pop(0) if ramp else None
        while k < len(rects):
            qb, i, w = rects[k]
            # matmul output must stay inside the tile and not straddle a
            # PSUM bank: require the running offset to be w-aligned
            if fd + w > ACT_FD or fd % w != 0:
                break
            grp.append((qb, i, w, fd))
            for jj in range(w // P):
                loc[(i, qb // P + jj)] = (len(groups), fd + jj * P)
            fd += w
            k += 1
            if limit is not None and len(grp) >= limit:
                break
        groups.append(grp)
    return groups, loc


def _build():
    EXPB, SQ4 = _register_dve_ops()
    nc = bacc.Bacc("TRN2", target_bir_lowering=False, debug=False,
                   num_devices=N_CORES)
    groups, loc = _plan()
    NG = len(groups)
    # Groups whose exp runs on the DVE instead of ScalarE. Measured: the PE
    # (scores + PV streams, ~70us incl per-matmul overheads) is jammed
    # right against ScalarE's exp stream (~66us) -- offloading exp only
    # shifts the bottleneck to the PE (90us run). Keep ScalarE-only.
    DVE_G = set()

    # pre0 = [kT tile 0 | q cols 0:128] packed host-side: 512B/partition
    # descriptors (the two pieces separately would be 256B descriptors,
    # below the SDMA line-rate threshold -- measured ~38GB/s vs ~170)
    pre0 = nc.dram_tensor("pre0", [P, 2 * P], F16, kind="ExternalInput")
    kT = nc.dram_tensor("kT", [P, KV], F16, kind="ExternalInput")
    qT = nc.dram_tensor("qT", [P, QTOT], F16, kind="ExternalInput")
    vaug = nc.dram_tensor("vaug", [P, NKV * VA], F16, kind="ExternalInput")
    # partition-major output: o[p, j*D + d] for q-chunk j
    o = nc.dram_tensor("o", [P, NCH * D], F32, kind="ExternalOutput")

    with tile.TileContext(nc) as tc:
        with (
            tc.tile_pool(name="const", bufs=1) as const_pool,
            tc.tile_pool(name="pT", bufs=12) as pT_pool,
            tc.tile_pool(name="expb", bufs=2) as expb_pool,
            tc.tile_pool(name="osb", bufs=3) as osb_pool,
            tc.tile_pool(name="recip", bufs=4) as recip_pool,
            tc.tile_pool(name="psumS", bufs=2, space="PSUM") as psumS_pool,
            tc.tile_pool(name="psumO", bufs=2, space="PSUM") as psumO_pool,
        ):
            # PE warmup: flips the HAM clock gate to 2.4GHz while the first
            # DMAs are in flight; sized to cover until the first data
            # matmul can run (~2us at the throttled 1.2GHz clock)
            # 9 x 256-col matmuls ~= 1.9us at the throttled 1.2GHz clock:
            # ends just as the first input DMA lands (~8.6us). More filler
            # matmuls are woven between the ramp groups below to keep the
            # HAM activity window hot while the ramp is DMA-bound.
            wa = const_pool.tile([P, 256], F16)
            nc.gpsimd.memset(wa[:], 0.0)
            c3t = const_pool.tile([P, 1], F32)
            nc.gpsimd.memset(c3t[:], EXPB_C3)
            wp = psumO_pool.tile([P, 256], F32, name="wp", tag="po")

            def filler(n):
                for _ in range(n):
                    nc.tensor.matmul(wp[:], wa[:, 0:P], wa[:], start=True,
                                     stop=True)

            filler(8)

            kT_sb = const_pool.tile([P, KV], F16)
            qT_sb = const_pool.tile([P, QTOT], F16)
            vaug_sb = const_pool.tile([P, NKV * VA], F16)

            # All input DMAs upfront, in need-order. The dynamic HWDGE ring
            # (qSPDynamicHW) executes one engine's dma_starts strictly FIFO
            # at ~200GB/s, so issue order IS arrival order -- no gating
            # needed, and the first (smallest) transfer isn't slowed by the
            # later ones. Arrival estimates in comments (first byte ~8.1us).
            pre0_sb = const_pool.tile([P, 2 * P], F16)
            nc.sync.dma_start(pre0_sb[:], pre0.ap())                      # ~8.5
            nc.sync.dma_start(qT_sb[:, P:512], qT.ap()[:, P:512])         # ~9.0
            # kT tiles 1-2 split out (512B descriptors, 64KB): phase A's
            # i=1 rects need kT1 ~2.5us before the rest of kT would land
            nc.sync.dma_start(kT_sb[:, P:3 * P], kT.ap()[:, P:3 * P])     # ~9.4
            nc.sync.dma_start(kT_sb[:, 3 * P:8 * P], kT.ap()[:, 3 * P:8 * P])  # ~10.3
            # vaug stays on the sync ring AFTER the ramp-critical kT/q: a
            # second HWDGE ring (nc.scalar.dma_start) was tried and lost --
            # both rings share the 16 SDMA engines, so it adds no aggregate
            # bandwidth and only lets vaug preempt the critical transfers.
            nc.sync.dma_start(vaug_sb[:, 0:8 * VA], vaug.ap()[:, 0:8 * VA])
            nc.sync.dma_start(kT_sb[:, 8 * P:], kT.ap()[:, 8 * P:])       # ~13.0
            nc.sync.dma_start(vaug_sb[:, 8 * VA:], vaug.ap()[:, 8 * VA:])
            nc.sync.dma_start(qT_sb[:, 512:1024], qT.ap()[:, 512:1024])
            nc.sync.dma_start(qT_sb[:, 1024:2048], qT.ap()[:, 1024:2048])
            nc.sync.dma_start(qT_sb[:, 2048:], qT.ap()[:, 2048:])

            # --- steady state ---
            pT_sbs = {}    # g -> pT tile
            osb_sbs = {}   # block -> tile
            po_cur = {}    # live po tiles keyed by chunk j

            # PV work queue: chunk-major, with evac + store milestones
            blk_of = {}    # chunk j -> (block, jlo, w)
            j0 = 0
            for b, nchunks in enumerate(STORE_BLK):
                for jl in range(nchunks):
                    blk_of[j0 + jl] = (b, j0, nchunks * P)
                j0 += nchunks
            pvq = []
            for j in range(NCH):
                for i in range(NKV):
                    pvq.append(("mm", j, i))
                pvq.append(("evac", j))
                b, jlo, w = blk_of[j]
                if j == jlo + w // P - 1:
                    pvq.append(("store", j))
            state = {"pos": 0, "g_emitted": 0}

            def pv_step(op):
                kind = op[0]
                if kind == "mm":
                    _, j, i = op
                    if i == 0:
                        po_cur[j] = psumO_pool.tile([P, VA], F32, name="po",
                                                    tag="po")
                        b, jlo, w = blk_of[j]
                        if b not in osb_sbs:
                            osb_sbs[b] = osb_pool.tile(
                                [P, w], F32, name="osb", tag="osb",
                                padded_shape=[P, 512])
                    g, off = loc[(i, j)]
                    nc.tensor.matmul(
                        po_cur[j][:],
                        pT_sbs[g][:, off:off + P],
                        vaug_sb[:, i * VA:(i + 1) * VA],
                        start=(i == 0), stop=(i == NKV - 1),
                        skip_group_check=True,
                    )
                elif kind == "evac":
                    _, j = op
                    po = po_cur.pop(j)
                    b, jlo, w = blk_of[j]
                    rc = recip_pool.tile([P, 1], F32, name="rc", tag="rc")
                    nc.vector.reciprocal(rc[:], po[:, D:D + 1])
                    nc.vector.tensor_scalar_mul(
                        osb_sbs[b][:, (j - jlo) * P:(j - jlo + 1) * P],
                        po[:, 0:D], rc[:])
                else:
                    _, j = op
                    b, jlo, w = blk_of[j]
                    nc.sync.dma_start(
                        o.ap()[:, jlo * D:jlo * D + w * (D // P)],
                        osb_sbs.pop(b)[:, 0:w])

            def ready(op):
                if op[0] != "mm":
                    return True
                _, j, i = op
                # RAMP_MIN_G: no PV before vaug's first half has landed
                # (~11.4us ~= act 5); a PV matmul waiting on DMA at the PE
                # queue head would starve the exp stream.
                if state["g_emitted"] < RAMP_MIN_G:
                    return False
                g_loc = loc[(i, j)][0]
                # lag shrinks toward the end so the final groups' PV work
                # overlaps the last activations instead of piling up after
                lag = min(LAG, NG - 1 - g_loc)
                return g_loc + lag < state["g_emitted"]

            def drain(cap):
                popped = 0
                while state["pos"] < len(pvq) and popped < cap:
                    op = pvq[state["pos"]]
                    if not ready(op):
                        break
                    pv_step(op)
                    state["pos"] += 1
                    if op[0] == "mm":
                        popped += 1

            for g, grp in enumerate(groups):
                fd = grp[-1][3] + grp[-1][2]
                ps = psumS_pool.tile([P, fd], F32, name="ps", tag="ps",
                                     padded_shape=[P, ACT_FD])
                for (qb, i, w, off) in grp:
                    kt = (pre0_sb[:, 0:P] if i == 0
                          else kT_sb[:, i * P:(i + 1) * P])
                    qs = (pre0_sb[:, P:2 * P] if qb == 0
                          else qT_sb[:, qb:qb + w])
                    nc.tensor.matmul(ps[:, off:off + w], kt, qs,
                                     start=True, stop=True,
                                     skip_group_check=True)
                    drain(cap=3)
                pT = pT_pool.tile([P, fd], F16, name="pT", tag="pT",
                                  padded_shape=[P, ACT_FD])
                if g in DVE_G:
                    # exp on the DVE: pass 1 (cubic base, frees the PSUM
                    # buf), evacs in between, pass 2 (^16 -> fp16 pT)
                    eb = expb_pool.tile([P, fd], F32, name="eb", tag="eb",
                                        padded_shape=[P, ACT_FD])
                    nc.vector._custom_dve(EXPB, out=eb[:], in0=ps[:],
                                          in1=c3t[:], s0=EXPB_C0,
                                          s1=EXPB_C1, imm2=EXPB_C2)
                    state["g_emitted"] = g + 1
                    drain(cap=6)
                    nc.vector._custom_dve(SQ4, out=pT[:], in0=eb[:])
                    pT_sbs[g] = pT
                    drain(cap=4)
                else:
                    nc.scalar.activation(pT[:], ps[:],
                                         mybir.ActivationFunctionType.Exp,
                                         scale=SCALE)
                    pT_sbs[g] = pT
                    state["g_emitted"] = g + 1
                    drain(cap=8)
                if g < 6:
                    # keep the HAM activity window hot through the DMA-bound
                    # ramp (3 per group measured best: larger bursts delay
                    # the next group's real matmuls when data lands mid-burst)
                    filler(3)
            while state["pos"] < len(pvq):
                pv_step(pvq[state["pos"]])
                state["pos"] += 1
    nc.compile()
    return nc


def _get_nc():
    if "nc" not in _CACHE:
        _CACHE["nc"] = _build()
    return _CACHE["nc"]


def kernel(query_states, key_states, value_states, attention_mask):
    # mask is all-ones by problem construction -> identity; ignored.
    q = np.asarray(query_states, dtype=np.float32).reshape(Q, H, D)
    k = np.asarray(key_states, dtype=np.float32).reshape(KV, D)
    v = np.asarray(value_states, dtype=np.float32).reshape(KV, D)

    kT = np.ascontiguousarray(k.T).astype(np.float16)  # [128, KV]
    # [V | ones] in fp16, laid out [128 kv-local, NKV * 129]
    va = np.concatenate(
        [v.reshape(NKV, P, D), np.ones((NKV, P, 1), np.float32)], axis=2
    ).astype(np.float16)
    vaug = np.ascontiguousarray(va.transpose(1, 0, 2)).reshape(P, NKV * VA)

    in_maps = []
    for c in range(N_CORES):
        qTc = np.empty((P, QTOT), np.float16)
        for hh in range(HPC):
            qTc[:, hh * Q:(hh + 1) * Q] = q[:, c * HPC + hh, :].T
        pre0 = np.ascontiguousarray(
            np.concatenate([kT[:, 0:P], qTc[:, 0:P]], axis=1))
        in_maps.append({"pre0": pre0, "qT": qTc, "kT": kT, "vaug": vaug})

    nc = _get_nc()
    res = run_bass_kernel_spmd(nc, in_maps, core_ids=list(range(N_CORES)))

    out = np.empty((Q, H, D), dtype=np.float32)
    for c in range(N_CORES):
        # o[p, j*D+d] -> q-major [QTOT, D] with q = j*128 + p
        oc = res.results[c]["o"].reshape(P, NCH, D).transpose(1, 0, 2)
        oc = oc.reshape(QTOT, D)
        for hh in range(HPC):
            out[:, c * HPC + hh, :] = oc[hh * Q:(hh + 1) * Q]
    return out.reshape(1, Q, H, D)


# revision 36
# speedup vs baseline: 1.0474x; 1.0474x over previous
"""MQA attention kernel for Trainium2, sharded over 8 NeuronCores.

Problem: query [1, 2048, 16, 128] f32, shared key/value [1, 2048, 128] f32,
mask [1, 16, 2048, 2048] bool (all ones -> no-op, per problem spec fill).

Sharding: tensor-parallel over heads, 2 heads per core; K/V replicated.

Per-core roofline: 65536 exp-elements/lane on ScalarE (54.6us at 1.2GHz) and
~131.6k matmul stream cycles on the PE (54.8us at 2.4GHz) -- co-critical.
The schedule keeps the ScalarE exp stream dense from ~9us to the end:

  - scores are computed as 128-col "rects" S^T[kv 128, q 128] (one fp16
    matmul each; 128-col matmuls sustain ~60ns incl hidden LDWEIGHTS).
    Rect order is chosen for DMA arrival + PV readiness:
      phase A: q[0:512] x kv tiles 0-7   (only needs the two upfront DMAs)
      phase B: q[0:512] x kv tiles 8-15  (needs kT tail, arrives ~12.4us)
      phase C: per 128-q chunk, all 16 kv tiles (chunk becomes PV-ready
               ~1.7us after its columns are exp'd -> small structural tail)
  - rects pack into [128, <=1536] PSUM tiles (3 banks, double-buffered);
    ONE ScalarE Exp per tile; ramp groups are small ([1,2,3,6] rects) so
    the exp stream starts as soon as the first 64KB DMA lands (~9us).
  - PV: out[q,0:128]+denominator in one PSUM accumulation group per 128-q
    chunk (lhsT = pT piece stationary, rhs = [V | ones] moving). PV pops
    are gated LAG=2 groups behind the exp stream (lag shrinking to 0 at
    the end): backlog drains into PE slack and overlaps the final acts.
  - input DMAs all issue upfront on the sync HWDGE ring in need-order;
    the ring starts transfers in issue order so the tiny act0 transfers
    aren't slowed by the bulk ones behind them.
  - 9 PE warmup matmuls + filler matmuls woven between the ramp groups
    keep the HAM activity window hot so the 2.4GHz clock gate opens
    ~10.5us in (it slipped to 18us without the fillers).
  - a custom-DVE exp path (EXPB cubic base + SQ4 squarings) is registered
    and was validated numerically, but left disabled (DVE_G = set()):
    the PE's scores+PV streams (~70us with per-matmul overheads) sit
    right against ScalarE's exp stream, so offloading exp just moves the
    bottleneck (measured 90us vs 81.7us ScalarE-only).

Host side: pre-transposes Q/K (free on CPU), casts to fp16, appends the
ones column to V, scatters per-core inputs, gathers per-core outputs.
"""

import numpy as np

import concourse.bass as bass
import concourse.tile as tile
from concourse import bacc, mybir
from concourse.bass_utils import run_bass_kernel_spmd

N_CORES = 8
H = 16
HPC = H // N_CORES   # heads per core
Q = 2048
KV = 2048
D = 128
P = 128
NKV = KV // P        # 16 kv tiles
VA = D + 1           # V augmented with a ones column
QTOT = HPC * Q       # q columns per core (across its heads)
NCH = QTOT // P      # 32 output q-chunks per core
SCALE = float(1.0 / np.sqrt(np.float32(D)))

ACT_FD = 1536        # one activation instruction per [128, ACT_FD] PSUM tile
RAMP_PACK = [1, 2, 3, 6]   # rects per act group during the ramp
LAG = 2              # PV pops trail the exp stream by this many groups
RAMP_MIN_G = 5       # no PV pops before this many acts are emitted

# store blocks, in 128-q chunks: the final block is a lone chunk so the
# last evac->store->receipt chain is as short as possible
STORE_BLK = [4, 4, 4, 4, 4, 4, 4, 3, 1]

F32 = mybir.dt.float32
F16 = mybir.dt.float16

# --- custom DVE exp -------------------------------------------------------
# exp(x*SCALE) ~= ((u+R*s)(u^2 + a*s*u + b*s^2))^16 with u = x*SCALE*s/16,
# s = 6^(-1/3) folding away the Taylor 1/6: the inner cubic is the factored
# degree-3 Taylor of e^u. Max rel err 1.2e-2 at |logit|=5.5, <1e-3 for
# |logit|<=3 (99.7% of N(0,1) mass); softmax-level impact ~3e-4.
# Runs as two DVE instructions: EXPB (6 ALU stages, PSUM->SBUF fp32) and
# SQ4 (4 stages of squaring, SBUF->SBUF fp16). This offloads ~20% of the
# exp stream from ScalarE (the kernel's critical path) to the idle DVE.
_DVE_S = 6.0 ** (-1.0 / 3.0)
EXPB_C0 = float(SCALE * _DVE_S / 16.0)
EXPB_C1 = 0.8783520721075015    # R*s
EXPB_C2 = 0.7726115523398118    # a*s
EXPB_C3 = 1.1384956349002726    # b*s^2

_CACHE = {}


def _register_dve_ops():
    if "dve" in _CACHE:
        return _CACHE["dve"]
    import concourse.dve_ops as dve_ops_mod
    from concourse.dve_ops import OPS, DveOp, get_dve_sub_opcode
    from concourse.dve_spec import (
        Spec, Src0, C0, C1, C2, C3, sq, lower, _spill_c3_to_src1, _has_src1,
    )
    from concourse.dve_uop import DveOpSpec

    existing = {op.name: op for op in OPS}
    if "ANT_EXPB" in existing:
        ops = (existing["ANT_EXPB"], existing["ANT_SQ4"])
    else:
        u = Src0 * C0
        body = _spill_c3_to_src1((u + C1) * (u * (u + C2) + C3))
        expb = DveOp("ANT_EXPB", Spec(body=body), subdim=False, uops_sha={})
        sq4 = DveOp("ANT_SQ4", Spec(body=sq(sq(sq(sq(Src0))))), subdim=False,
                    uops_sha={})
        for op in (expb, sq4):
            OPS.append(op)
            dve_ops_mod._SUB_OPCODE_FOR_NAME[op.name] = (
                dve_ops_mod._CUSTOM_DVE_ROW_BASE + len(OPS) - 1)
            assert dve_ops_mod._SUB_OPCODE_FOR_NAME[op.name] < 0x20
            for ver in ("v3", "v4"):
                compiled = DveOpSpec(
                    name=op.name, opcode=get_dve_sub_opcode(op.name),
                    uops=lower(op.spec, ver=ver), rd1_en=_has_src1(op.spec))
                op.uops_sha[ver] = compiled.sha(ver)
        ops = (expb, sq4)
    _CACHE["dve"] = ops
    return ops


def _plan():
    """Static schedule.

    Returns:
      groups: list of act groups; each is a list of rects (qb, i, off) with
              off the rect's column offset inside the group's PSUM tile.
      loc:    (i, chunk j) -> (g, off) location of that pT piece.
      """
    rects = []  # (qb, i)
    # phase A1: i 0..1 x q[0:512] (kT tiles 0-1 + q[0:512] arrive first)
    for i in range(2):
        for qb in range(0, 512, P):
            rects.append((qb, i))
    # phase A2: i 2..7 x q[0:512]
    for i in range(2, 8):
        for qb in range(0, 512, P):
            rects.append((qb, i))
    # phase B: qb-major so chunks 0-3 become PV-complete early and
    # staggered (avoids a mid-stream PV pileup); kT tail lands ~11.5us
    for qb in range(0, 512, P):
        for i in range(8, 16):
            rects.append((qb, i))
    # phase C: remaining q, full i sweep per 128-q chunk
    for qb in range(512, QTOT, P):
        for i in range(NKV):
            rects.append((qb, i))
    assert len(rects) == (QTOT // P) * NKV

    groups = []
    loc = {}
    k = 0
    ramp = list(RAMP_PACK)
    while k < len(rects):
        n = ramp.pop(0) if ramp else ACT_FD // P
        n = min(n, len(rects) - k)
        grp = []
        for m in range(n):
            qb, i = rects[k + m]
            off = m * P
            grp.append((qb, i, off))
            loc[(i, qb // P)] = (len(groups), off)
        groups.append(grp)
        k += n
    return groups, loc


def _build():
    EXPB, SQ4 = _register_dve_ops()
    nc = bacc.Bacc("TRN2", target_bir_lowering=False, debug=False,
                   num_devices=N_CORES)
    groups, loc = _plan()
    NG = len(groups)
    # Groups whose exp runs on the DVE instead of ScalarE. Measured: the PE
    # (scores + PV streams, ~70us incl per-matmul overheads) is jammed
    # right against ScalarE's exp stream (~66us) -- offloading exp only
    # shifts the bottleneck to the PE (90us run). Keep ScalarE-only.
    DVE_G = set()

    # pre0 = [kT tile 0 | q cols 0:128] packed host-side: 512B/partition
    # descriptors (the two pieces separately would be 256B descriptors,
    # below the SDMA line-rate threshold -- measured ~38GB/s vs ~170)
    pre0 = nc.dram_tensor("pre0", [P, 2 * P], F16, kind="ExternalInput")
    kT = nc.dram_tensor("kT", [P, KV], F16, kind="ExternalInput")
    qT = nc.dram_tensor("qT", [P, QTOT], F16, kind="ExternalInput")
    vaug = nc.dram_tensor("vaug", [P, NKV * VA], F16, kind="ExternalInput")
    # partition-major output: o[p, j*D + d] for q-chunk j
    o = nc.dram_tensor("o", [P, NCH * D], F32, kind="ExternalOutput")

    with tile.TileContext(nc) as tc:
        with (
            tc.tile_pool(name="const", bufs=1) as const_pool,
            tc.tile_pool(name="pT", bufs=12) as pT_pool,
            tc.tile_pool(name="expb", bufs=2) as expb_pool,
            tc.tile_pool(name="osb", bufs=3) as osb_pool,
            tc.tile_pool(name="recip", bufs=4) as recip_pool,
            tc.tile_pool(name="psumS", bufs=2, space="PSUM") as psumS_pool,
            tc.tile_pool(name="psumO", bufs=2, space="PSUM") as psumO_pool,
        ):
            # PE warmup: flips the HAM clock gate to 2.4GHz while the first
            # DMAs are in flight; sized to cover until the first data
            # matmul can run (~2us at the throttled 1.2GHz clock)
            # 9 x 256-col matmuls ~= 1.9us at the throttled 1.2GHz clock:
            # ends just as the first input DMA lands (~8.6us). More filler
            # matmuls are woven between the ramp groups below to keep the
            # HAM activity window hot while the ramp is DMA-bound.
            wa = const_pool.tile([P, 256], F16)
            nc.gpsimd.memset(wa[:], 0.0)
            c3t = const_pool.tile([P, 1], F32)
            nc.gpsimd.memset(c3t[:], EXPB_C3)
            wp = psumO_pool.tile([P, 256], F32, name="wp", tag="po")

            def filler(n):
                for _ in range(n):
                    nc.tensor.matmul(wp[:], wa[:, 0:P], wa[:], start=True,
                                     stop=True)

            filler(8)

            kT_sb = const_pool.tile([P, KV], F16)
            qT_sb = const_pool.tile([P, QTOT], F16)
            vaug_sb = const_pool.tile([P, NKV * VA], F16)

            # All input DMAs upfront, in need-order. The dynamic HWDGE ring
            # (qSPDynamicHW) executes one engine's dma_starts strictly FIFO
            # at ~200GB/s, so issue order IS arrival order -- no gating
            # needed, and the first (smallest) transfer isn't slowed by the
            # later ones. Arrival estimates in comments (first byte ~8.1us).
            pre0_sb = const_pool.tile([P, 2 * P], F16)
            nc.sync.dma_start(pre0_sb[:], pre0.ap())                      # ~8.5
            nc.sync.dma_start(qT_sb[:, P:512], qT.ap()[:, P:512])         # ~9.0
            # kT tiles 1-2 split out (512B descriptors, 64KB): phase A's
            # i=1 rects need kT1 ~2.5us before the rest of kT would land
            nc.sync.dma_start(kT_sb[:, P:3 * P], kT.ap()[:, P:3 * P])     # ~9.4
            nc.sync.dma_start(kT_sb[:, 3 * P:8 * P], kT.ap()[:, 3 * P:8 * P])  # ~10.3
            # vaug stays on the sync ring AFTER the ramp-critical kT/q: a
            # second HWDGE ring (nc.scalar.dma_start) was tried and lost --
            # both rings share the 16 SDMA engines, so it adds no aggregate
            # bandwidth and only lets vaug preempt the critical transfers.
            nc.sync.dma_start(vaug_sb[:, 0:8 * VA], vaug.ap()[:, 0:8 * VA])
            nc.sync.dma_start(kT_sb[:, 8 * P:], kT.ap()[:, 8 * P:])       # ~13.0
            nc.sync.dma_start(vaug_sb[:, 8 * VA:], vaug.ap()[:, 8 * VA:])
            nc.sync.dma_start(qT_sb[:, 512:1024], qT.ap()[:, 512:1024])
            nc.sync.dma_start(qT_sb[:, 1024:2048], qT.ap()[:, 1024:2048])
            nc.sync.dma_start(qT_sb[:, 2048:], qT.ap()[:, 2048:])

            # --- steady state ---
            pT_sbs = {}    # g -> pT tile
            osb_sbs = {}   # block -> tile
            po_cur = {}    # live po tiles keyed by chunk j

            # PV work queue: chunk-major, with evac + store milestones
            blk_of = {}    # chunk j -> (block, jlo, w)
            j0 = 0
            for b, nchunks in enumerate(STORE_BLK):
                for jl in range(nchunks):
                    blk_of[j0 + jl] = (b, j0, nchunks * P)
                j0 += nchunks
            pvq = []
            for j in range(NCH):
                for i in range(NKV):
                    pvq.append(("mm", j, i))
                pvq.append(("evac", j))
                b, jlo, w = blk_of[j]
                if j == jlo + w // P - 1:
                    pvq.append(("store", j))
            state = {"pos": 0, "g_emitted": 0}

            def pv_step(op):
                kind = op[0]
                if kind == "mm":
                    _, j, i = op
                    if i == 0:
                        po_cur[j] = psumO_pool.tile([P, VA], F32, name="po",
                                                    tag="po")
                        b, jlo, w = blk_of[j]
                        if b not in osb_sbs:
                            osb_sbs[b] = osb_pool.tile(
                                [P, w], F32, name="osb", tag="osb",
                                padded_shape=[P, 512])
                    g, off = loc[(i, j)]
                    nc.tensor.matmul(
                        po_cur[j][:],
                        pT_sbs[g][:, off:off + P],
                        vaug_sb[:, i * VA:(i + 1) * VA],
                        start=(i == 0), stop=(i == NKV - 1),
                        skip_group_check=True,
                    )
                elif kind == "evac":
                    _, j = op
                    po = po_cur.pop(j)
                    b, jlo, w = blk_of[j]
                    rc = recip_pool.tile([P, 1], F32, name="rc", tag="rc")
                    nc.vector.reciprocal(rc[:], po[:, D:D + 1])
                    nc.vector.tensor_scalar_mul(
                        osb_sbs[b][:, (j - jlo) * P:(j - jlo + 1) * P],
                        po[:, 0:D], rc[:])
                else:
                    _, j = op
                    b, jlo, w = blk_of[j]
                    nc.sync.dma_start(
                        o.ap()[:, jlo * D:jlo * D + w * (D // P)],
                        osb_sbs.pop(b)[:, 0:w])

            def ready(op):
                if op[0] != "mm":
                    return True
                _, j, i = op
                # RAMP_MIN_G: no PV before vaug's first half has landed
                # (~11.4us ~= act 5); a PV matmul waiting on DMA at the PE
                # queue head would starve the exp stream.
                if state["g_emitted"] < RAMP_MIN_G:
                    return False
                g_loc = loc[(i, j)][0]
                # lag shrinks toward the end so the final groups' PV work
                # overlaps the last activations instead of piling up after
                lag = min(LAG, NG - 1 - g_loc)
                return g_loc + lag < state["g_emitted"]

            def drain(cap):
                popped = 0
                while state["pos"] < len(pvq) and popped < cap:
                    op = pvq[state["pos"]]
                    if not ready(op):
                        break
                    pv_step(op)
                    state["pos"] += 1
                    if op[0] == "mm":
                        popped += 1

            for g, grp in enumerate(groups):
                fd = len(grp) * P
                ps = psumS_pool.tile([P, fd], F32, name="ps", tag="ps",
                                     padded_shape=[P, ACT_FD])
                for (qb, i, off) in grp:
                    kt = (pre0_sb[:, 0:P] if i == 0
                          else kT_sb[:, i * P:(i + 1) * P])
                    qs = (pre0_sb[:, P:2 * P] if qb == 0
                          else qT_sb[:, qb:qb + P])
                    nc.tensor.matmul(ps[:, off:off + P], kt, qs,
                                     start=True, stop=True,
                                     skip_group_check=True)
                    drain(cap=3)
                pT = pT_pool.tile([P, fd], F16, name="pT", tag="pT",
                                  padded_shape=[P, ACT_FD])
                if g in DVE_G:
                    # exp on the DVE: pass 1 (cubic base, frees the PSUM
                    # buf), evacs in between, pass 2 (^16 -> fp16 pT)
                    eb = expb_pool.tile([P, fd], F32, name="eb", tag="eb",
                                        padded_shape=[P, ACT_FD])
                    nc.vector._custom_dve(EXPB, out=eb[:], in0=ps[:],
                                          in1=c3t[:], s0=EXPB_C0,
                                          s1=EXPB_C1, imm2=EXPB_C2)
                    state["g_emitted"] = g + 1
                    drain(cap=6)
                    nc.vector._custom_dve(SQ4, out=pT[:], in0=eb[:])
                    pT_sbs[g] = pT
                    drain(cap=4)
                else:
                    nc.scalar.activation(pT[:], ps[:],
                                         mybir.ActivationFunctionType.Exp,
                                         scale=SCALE)
                    pT_sbs[g] = pT
                    state["g_emitted"] = g + 1
                    drain(cap=8)
                if g < 6:
                    # keep the HAM activity window hot through the DMA-bound
                    # ramp (3 per group measured best: larger bursts delay
                    # the next group's real matmuls when data lands mid-burst)
                    filler(3)
            while state["pos"] < len(pvq):
                pv_step(pvq[state["pos"]])
                state["pos"] += 1
    nc.compile()
    return nc


def _get_nc():
    if "nc" not in _CACHE:
        _CACHE["nc"] = _build()
    return _CACHE["nc"]


def kernel(query_states, key_states, value_states, attention_mask):
    # mask is all-ones by problem construction -> identity; ignored.
    q = np.asarray(query_states, dtype=np.float32).reshape(Q, H, D)
    k = np.asarray(key_states, dtype=np.float32).reshape(KV, D)
    v = np.asarray(value_states, dtype=np.float32).reshape(KV, D)

    kT = np.ascontiguousarray(k.T).astype(np.float16)  # [128, KV]
    # [V | ones] in fp16, laid out [128 kv-local, NKV * 129]
    va = np.concatenate(
        [v.reshape(NKV, P, D), np.ones((NKV, P, 1), np.float32)], axis=2
    ).astype(np.float16)
    vaug = np.ascontiguousarray(va.transpose(1, 0, 2)).reshape(P, NKV * VA)

    in_maps = []
    for c in range(N_CORES):
        qTc = np.empty((P, QTOT), np.float16)
        for hh in range(HPC):
            qTc[:, hh * Q:(hh + 1) * Q] = q[:, c * HPC + hh, :].T
        pre0 = np.ascontiguousarray(
            np.concatenate([kT[:, 0:P], qTc[:, 0:P]], axis=1))
        in_maps.append({"pre0": pre0, "qT": qTc, "kT": kT, "vaug": vaug})

    nc = _get_nc()
    res = run_bass_kernel_spmd(nc, in_maps, core_ids=list(range(N_CORES)))

    out = np.empty((Q, H, D), dtype=np.float32)
    for c in range(N_CORES):
        # o[p, j*D+d] -> q-major [QTOT, D] with q = j*128 + p
        oc = res.results[c]["o"].reshape(P, NCH, D).transpose(1, 0, 2)
        oc = oc.reshape(QTOT, D)
        for hh in range(HPC):
            out[:, c * HPC + hh, :] = oc[hh * Q:(hh + 1) * Q]
    return out.reshape(1, Q, H, D)
